# revision 10
# baseline (speedup 1.0000x reference)
"""Trainium2 Bass kernel for nn_BlockAttnRes (fused RMSNorm-softmax pooling).

Reference computation (all fp32):
    V = concat([blocks, partial[None]], axis=0)          # (8, B, T, D)
    K = V * rsqrt(mean(V^2, -1) + eps) * norm_weight
    logits  = einsum('d,nbtd->nbt', w, K)
    weights = softmax(logits, axis=0)                    # over the 8 sources
    out     = einsum('nbt,nbtd->btd', weights, V)        # (B, T, D)

Key algebraic reduction: K never needs materializing.
    logits[n,tok] = (sum_d wn[d]*V[n,tok,d]) * rsqrt(mean_d(V^2) + eps)
with wn = w * norm_weight (folded on host). So per (source, token) we need
two free-axis reductions over D, an 8-way softmax, and a weighted sum --
a single pass over V from HBM (memory-roofline).

Sharding: data-parallel over the 8192 tokens (B*T) across 8 NeuronCores,
1024 tokens each, no cross-core communication.

Per-core engine split (per 128-token tile, 8 sources):
  ACT  : Square+accumulate -> s2[:,n];  Exp+accumulate -> e, denom;
         diagonal-weight tiles for the PE path (Copy with per-partition scale)
  DVE  : fused multiply+reduce (tensor_tensor_reduce) -> dot[:,n];
         rsqrt via bit-trick seed + Newton; softmax smalls;
         tail of the weighted sum via fused scalar_tensor_tensor FMA chain
  PE   : bulk of the weighted sum: out += diag(wgt_n) @ V_n accumulated in
         PSUM (fp32, full precision)
  DMA  : 8x512KB loads + 1x512KB store per tile (nc.sync, HWDGE)
"""

import os
import sys

import numpy as np

sys.path.insert(0, "/opt/trn_rl_repo")

N_BLOCKS, B, T, D = 7, 4, 2048, 1024
N_SRC = N_BLOCKS + 1          # 8 sources after appending `partial`
N_CORES = 8
TOK_TOTAL = B * T             # 8192
TOK_PER_CORE = TOK_TOTAL // N_CORES   # 1024
P = 128                       # SBUF partitions / tokens per tile
TILES_PER_CORE = TOK_PER_CORE // P    # 8
EPS = float(np.finfo(np.float32).eps)
MAGIC_P1 = 0x5F3759DF + 1     # rsqrt bit-trick constant (+1 for xor-negate form)

# number of sources whose weighted-sum pass runs on the TensorEngine
# (diag-matmul into PSUM); the rest run on DVE as a fused FMA chain.
N_PE = int(os.environ.get("KERNEL_N_PE", "6"))

_STATE: dict = {}


def _split_multi_waits(nc):
    """TPB instructions encode a single sem-wait; this walrus build refuses
    instructions carrying more (`Too many sync wait commands`). Split extra
    waits onto single-wait NoOps on the same engine, preserving per-engine
    program order (and therefore semantics)."""
    import concourse.mybir as mybir

    for fn in nc.m.functions:
        for blk in fn.blocks:
            insts = list(blk.instructions)
            out = []
            changed = False
            for ins in insts:
                si = ins.sync_info
                if si is not None and len(si.on_wait) > 1:
                    waits = list(si.on_wait)
                    for k, w in enumerate(waits[:-1]):
                        nop = mybir.InstNoOp(name=f"{ins.name}-sw{k}", ins=[], outs=[])
                        nop.engine = ins.engine
                        nop.sync_info = mybir.SyncInfo(on_wait=[w], on_update=[])
                        out.append(nop)
                    ins.sync_info = mybir.SyncInfo(
                        on_wait=[waits[-1]], on_update=list(si.on_update)
                    )
                    changed = True
                out.append(ins)
            if changed:
                blk.instructions = out
    return nc


def _build_nc(n_pe: int, repeat: int = 1):
    import concourse.bass as bass
    import concourse.mybir as mybir
    import concourse.tile as tile
    from contextlib import ExitStack

    f32 = mybir.dt.float32
    i32 = mybir.dt.int32
    Alu = mybir.AluOpType
    Act = mybir.ActivationFunctionType

    nc = bass.Bass("TRN2", target_bir_lowering=False, debug=False)

    blocks_d = nc.dram_tensor(
        "blocks", [N_BLOCKS, TOK_PER_CORE, D], f32, kind="ExternalInput"
    )
    partial_d = nc.dram_tensor("partial", [TOK_PER_CORE, D], f32, kind="ExternalInput")
    wn_d = nc.dram_tensor("wn", [P, D], f32, kind="ExternalInput")
    ident_d = nc.dram_tensor("ident", [P, P], f32, kind="ExternalInput")
    out_d = nc.dram_tensor("out", [TOK_PER_CORE, D], f32, kind="ExternalOutput")

    bap = blocks_d.ap()
    pap = partial_d.ap()
    oap = out_d.ap()

    pe_src = list(range(N_SRC - n_pe, N_SRC))   # sources on the PE path
    dve_src = list(range(0, N_SRC - n_pe))      # sources on the DVE FMA chain

    with tile.TileContext(nc) as tc, ExitStack() as ctx:
        const_pool = ctx.enter_context(tc.tile_pool(name="const", bufs=1))
        vpool = ctx.enter_context(tc.tile_pool(name="v", bufs=3))
        scr_pool = ctx.enter_context(tc.tile_pool(name="scr", bufs=2))
        stat_pool = ctx.enter_context(tc.tile_pool(name="stat", bufs=3))
        diag_pool = ctx.enter_context(tc.tile_pool(name="diag", bufs=2))
        out_pool = ctx.enter_context(tc.tile_pool(name="outp", bufs=3))
        psum_pool = ctx.enter_context(tc.tile_pool(name="ps", bufs=2, space="PSUM"))

        wn_sb = const_pool.tile([P, D], f32, name="wn_sb")
        nc.sync.dma_start(wn_sb[:], wn_d.ap()[:, :])
        if n_pe:
            ident_sb = const_pool.tile([P, P], f32, name="ident_sb")
            nc.sync.dma_start(ident_sb[:], ident_d.ap()[:, :])

        def tile_body(t):
            sl = slice(t * P, (t + 1) * P)

            v = []
            for n in range(N_BLOCKS):
                vt = vpool.tile([P, D], f32, tag=f"v{n}", name=f"v{n}_{t}")
                nc.sync.dma_start(vt[:], bap[n, sl, :])
                v.append(vt)
            vt = vpool.tile([P, D], f32, tag="v7", name=f"v7_{t}")
            nc.sync.dma_start(vt[:], pap[sl, :])
            v.append(vt)

            s2 = stat_pool.tile([P, N_SRC], f32, tag="s2", name=f"s2_{t}")
            dot = stat_pool.tile([P, N_SRC], f32, tag="dot", name=f"dot_{t}")
            for n in range(N_SRC):
                sq_scr = scr_pool.tile([P, D], f32, tag="sq", name=f"sq_{t}_{n}")
                nc.scalar.activation(
                    sq_scr[:], v[n][:], Act.Square, accum_out=s2[:, n : n + 1]
                )
                pr_scr = scr_pool.tile([P, D], f32, tag="pr", name=f"pr_{t}_{n}")
                nc.vector.scalar_tensor_tensor(
                    out=pr_scr[:],
                    in0=v[n][:],
                    scalar=1.0,
                    in1=wn_sb[:],
                    op0=Alu.mult,
                    op1=Alu.mult,
                    accum_out=dot[:, n : n + 1],
                )

            # ms = s2/D + eps ; rs = rsqrt(ms) via bit trick + 3 Newton steps
            ms = stat_pool.tile([P, N_SRC], f32, tag="ms", name=f"ms_{t}")
            nc.vector.tensor_scalar(ms[:], s2[:], 1.0 / D, EPS, Alu.mult, Alu.add)
            ti = stat_pool.tile([P, N_SRC], i32, tag="ti", name=f"ti_{t}")
            nc.vector.tensor_single_scalar(
                ti[:], ms[:].bitcast(i32), 1, Alu.logical_shift_right
            )
            y = stat_pool.tile([P, N_SRC], f32, tag="y", name=f"y_{t}")
            # y_bits = MAGIC - ti  ==  (ti ^ -1) + (MAGIC + 1); walrus refuses
            # mixed bitwise+arith in one tensor_scalar, so two instructions.
            nc.vector.tensor_single_scalar(ti[:], ti[:], -1, Alu.bitwise_xor)
            nc.vector.tensor_single_scalar(y[:].bitcast(i32), ti[:], MAGIC_P1, Alu.add)
            for it in range(3):
                a = stat_pool.tile([P, N_SRC], f32, tag="nra", name=f"nra_{t}_{it}")
                nc.vector.tensor_tensor(a[:], y[:], y[:], Alu.mult)
                nc.vector.tensor_tensor(a[:], a[:], ms[:], Alu.mult)
                nc.vector.tensor_scalar(a[:], a[:], -0.5, 1.5, Alu.mult, Alu.add)
                nc.vector.tensor_tensor(y[:], y[:], a[:], Alu.mult)

            lg = stat_pool.tile([P, N_SRC], f32, tag="lg", name=f"lg_{t}")
            nc.vector.tensor_tensor(lg[:], dot[:], y[:], Alu.mult)

            nm = stat_pool.tile([P, 1], f32, tag="nm", name=f"nm_{t}")
            nc.vector.tensor_reduce(
                nm[:], lg[:], axis=mybir.AxisListType.X, op=Alu.max, negate=True
            )
            e = stat_pool.tile([P, N_SRC], f32, tag="e", name=f"e_{t}")
            den = stat_pool.tile([P, 1], f32, tag="den", name=f"den_{t}")
            nc.scalar.activation(e[:], lg[:], Act.Exp, bias=nm[:], accum_out=den[:])
            r = stat_pool.tile([P, 1], f32, tag="r", name=f"r_{t}")
            nc.vector.reciprocal(r[:], den[:])
            wgt = stat_pool.tile([P, N_SRC], f32, tag="wgt", name=f"wgt_{t}")
            nc.vector.tensor_single_scalar(wgt[:], e[:], r[:], Alu.mult)

            # ---- weighted sum ----
            acc = out_pool.tile([P, D], f32, tag="acc", name=f"acc_{t}")

            if n_pe:
                diags = []
                for j, n in enumerate(pe_src):
                    dg = diag_pool.tile([P, P], f32, tag=f"dg{j}", name=f"dg{j}_{t}")
                    nc.scalar.activation(
                        dg[:], ident_sb[:], Act.Copy, scale=wgt[:, n : n + 1]
                    )
                    diags.append(dg)
                ps = psum_pool.tile([P, D], f32, tag="ps", name=f"ps_{t}")
                half = D // 2
                for h in range(2):
                    cs = slice(h * half, (h + 1) * half)
                    for j, n in enumerate(pe_src):
                        nc.tensor.matmul(
                            ps[:, cs],
                            lhsT=diags[j][:],
                            rhs=v[n][:, cs],
                            start=(j == 0),
                            stop=(j == len(pe_src) - 1),
                        )
                # first DVE FMA consumes the PSUM accumulation
                n0 = dve_src[0]
                nc.vector.scalar_tensor_tensor(
                    out=acc[:],
                    in0=v[n0][:],
                    scalar=wgt[:, n0 : n0 + 1],
                    in1=ps[:],
                    op0=Alu.mult,
                    op1=Alu.add,
                )
                rest = dve_src[1:]
            else:
                n0 = dve_src[0]
                nc.vector.tensor_single_scalar(
                    acc[:], v[n0][:], wgt[:, n0 : n0 + 1], Alu.mult
                )
                rest = dve_src[1:]

            for n in rest:
                nc.vector.scalar_tensor_tensor(
                    out=acc[:],
                    in0=v[n][:],
                    scalar=wgt[:, n : n + 1],
                    in1=acc[:],
                    op0=Alu.mult,
                    op1=Alu.add,
                )

            nc.sync.dma_start(oap[sl, :], acc[:])

        if repeat == 1:
            for t in range(TILES_PER_CORE):
                tile_body(t)
        else:
            # benchmark mode: re-run the whole per-core computation `repeat`
            # times inside a hardware loop so marginal wall-clock isolates
            # on-device execution time from axon/PJRT dispatch overhead.
            with tc.For_i(0, repeat, 1):
                for t in range(TILES_PER_CORE):
                    tile_body(t)

    return _split_multi_waits(nc)


def _get_state():
    if "nc" not in _STATE:
        _STATE["nc"] = _build_nc(N_PE)
    return _STATE["nc"]


def _prepare_in_maps(blocks, partial, norm_weight, w):
    blocks = np.asarray(blocks, dtype=np.float32)
    partial = np.asarray(partial, dtype=np.float32)
    norm_weight = np.asarray(norm_weight, dtype=np.float32)
    w = np.asarray(w, dtype=np.float32)

    wn = (w * norm_weight).astype(np.float32)
    wn_b = np.ascontiguousarray(np.broadcast_to(wn, (P, D)))
    ident = np.eye(P, dtype=np.float32)

    blocks_f = blocks.reshape(N_BLOCKS, TOK_TOTAL, D)
    partial_f = partial.reshape(TOK_TOTAL, D)

    in_maps = []
    for c in range(N_CORES):
        sl = slice(c * TOK_PER_CORE, (c + 1) * TOK_PER_CORE)
        in_maps.append(
            {
                "blocks": np.ascontiguousarray(blocks_f[:, sl, :]),
                "partial": np.ascontiguousarray(partial_f[sl, :]),
                "wn": wn_b,
                "ident": ident,
            }
        )
    return in_maps


def _run(inputs, trace=False, **kwargs):
    from concourse.bass_utils import run_bass_kernel_spmd

    nc = _get_state()
    in_maps = _prepare_in_maps(**inputs)
    bkr = run_bass_kernel_spmd(
        nc, in_maps, core_ids=list(range(N_CORES)), trace=trace, **kwargs
    )
    out = np.concatenate([bkr.results[c]["out"] for c in range(N_CORES)], axis=0)
    return out.reshape(B, T, D), bkr


def kernel(**inputs) -> np.ndarray:
    out, _ = _run(inputs, trace=False)
    return out


# revision 13
# speedup vs baseline: 2.5241x; 2.5241x over previous
"""Trainium2 Bass kernel for nn_BlockAttnRes (fused RMSNorm-softmax pooling).

Reference computation (all fp32):
    V = concat([blocks, partial[None]], axis=0)          # (8, B, T, D)
    K = V * rsqrt(mean(V^2, -1) + eps) * norm_weight
    logits  = einsum('d,nbtd->nbt', w, K)
    weights = softmax(logits, axis=0)                    # over the 8 sources
    out     = einsum('nbt,nbtd->btd', weights, V)        # (B, T, D)

Key algebraic reduction: K never needs materializing.
    logits[n,tok] = (sum_d wn[d]*V[n,tok,d]) * rsqrt(mean_d(V^2) + eps)
with wn = w * norm_weight (folded on host). So per (source, token) we need
two free-axis reductions over D, an 8-way softmax, and a weighted sum --
a single pass over V from HBM (memory-roofline).

Sharding: data-parallel over the 8192 tokens (B*T) across 8 NeuronCores,
1024 tokens each, no cross-core communication.

Per-core engine split (per 128-token tile, 8 sources):
  ACT  : Square+accumulate -> s2[:,n];  Exp+accumulate -> e, denom;
         diagonal-weight tiles for the PE path (Copy with per-partition scale)
  DVE  : fused multiply+reduce (tensor_tensor_reduce) -> dot[:,n];
         rsqrt via bit-trick seed + Newton; softmax smalls;
         tail of the weighted sum via fused scalar_tensor_tensor FMA chain
  PE   : bulk of the weighted sum: out += diag(wgt_n) @ V_n accumulated in
         PSUM (fp32, full precision)
  DMA  : 8x512KB loads + 1x512KB store per tile (nc.sync, HWDGE)
"""

import os
import sys

import numpy as np

sys.path.insert(0, "/opt/trn_rl_repo")

N_BLOCKS, B, T, D = 7, 4, 2048, 1024
N_SRC = N_BLOCKS + 1          # 8 sources after appending `partial`
N_CORES = 8
TOK_TOTAL = B * T             # 8192
TOK_PER_CORE = TOK_TOTAL // N_CORES   # 1024
P = 128                       # SBUF partitions / tokens per tile
TILES_PER_CORE = TOK_PER_CORE // P    # 8
EPS = float(np.finfo(np.float32).eps)
MAGIC_P1 = 0x5F3759DF + 1     # rsqrt bit-trick constant (+1 for xor-negate form)

# number of sources whose weighted-sum pass runs on the TensorEngine
# (diag-matmul into PSUM); the rest run on DVE as a fused FMA chain.
N_PE = int(os.environ.get("KERNEL_N_PE", "6"))

_STATE: dict = {}


def _split_multi_waits(nc):
    """TPB instructions encode a single sem-wait; this walrus build refuses
    instructions carrying more (`Too many sync wait commands`). Split extra
    waits onto single-wait NoOps on the same engine, preserving per-engine
    program order (and therefore semantics)."""
    import concourse.mybir as mybir

    for fn in nc.m.functions:
        for blk in fn.blocks:
            insts = list(blk.instructions)
            out = []
            changed = False
            for ins in insts:
                si = ins.sync_info
                if si is not None and len(si.on_wait) > 1:
                    waits = list(si.on_wait)
                    for k, w in enumerate(waits[:-1]):
                        nop = mybir.InstNoOp(name=f"{ins.name}-sw{k}", ins=[], outs=[])
                        nop.engine = ins.engine
                        nop.sync_info = mybir.SyncInfo(on_wait=[w], on_update=[])
                        out.append(nop)
                    ins.sync_info = mybir.SyncInfo(
                        on_wait=[waits[-1]], on_update=list(si.on_update)
                    )
                    changed = True
                out.append(ins)
            if changed:
                blk.instructions = out
    return nc


def _build_nc(n_pe: int, repeat: int = 1, loop: bool = True, mode: str = "full"):
    import concourse.bass as bass
    import concourse.mybir as mybir
    import concourse.tile as tile
    from contextlib import ExitStack

    f32 = mybir.dt.float32
    i32 = mybir.dt.int32
    Alu = mybir.AluOpType
    Act = mybir.ActivationFunctionType

    nc = bass.Bass("TRN2", target_bir_lowering=False, debug=False)

    blocks_d = nc.dram_tensor(
        "blocks", [N_BLOCKS, TOK_PER_CORE, D], f32, kind="ExternalInput"
    )
    partial_d = nc.dram_tensor("partial", [TOK_PER_CORE, D], f32, kind="ExternalInput")
    wn_d = nc.dram_tensor("wn", [P, D], f32, kind="ExternalInput")
    ident_d = nc.dram_tensor("ident", [P, P], f32, kind="ExternalInput")
    out_d = nc.dram_tensor("out", [TOK_PER_CORE, D], f32, kind="ExternalOutput")

    bap = blocks_d.ap()
    pap = partial_d.ap()
    oap = out_d.ap()

    pe_src = list(range(N_SRC - n_pe, N_SRC))   # sources on the PE path
    dve_src = list(range(0, N_SRC - n_pe))      # sources on the DVE FMA chain

    with tile.TileContext(nc) as tc, ExitStack() as ctx:
        const_pool = ctx.enter_context(tc.tile_pool(name="const", bufs=1))
        vpool = ctx.enter_context(tc.tile_pool(name="v", bufs=3))
        scr_pool = ctx.enter_context(tc.tile_pool(name="scr", bufs=2))
        stat_pool = ctx.enter_context(tc.tile_pool(name="stat", bufs=3))
        diag_pool = ctx.enter_context(tc.tile_pool(name="diag", bufs=2))
        out_pool = ctx.enter_context(tc.tile_pool(name="outp", bufs=3))
        psum_pool = ctx.enter_context(tc.tile_pool(name="ps", bufs=2, space="PSUM"))

        wn_sb = const_pool.tile([P, D], f32, name="wn_sb")
        nc.sync.dma_start(wn_sb[:], wn_d.ap()[:, :])
        if n_pe:
            ident_sb = const_pool.tile([P, P], f32, name="ident_sb")
            nc.sync.dma_start(ident_sb[:], ident_d.ap()[:, :])

        def tile_body(t, r=0):
            sl = slice(t * P, (t + 1) * P)

            v = []
            for n in range(N_BLOCKS):
                vt = vpool.tile([P, D], f32, tag=f"v{n}", name=f"v{n}_{t}_{r}")
                nc.sync.dma_start(vt[:], bap[n, sl, :])
                v.append(vt)
            vt = vpool.tile([P, D], f32, tag="v7", name=f"v7_{t}_{r}")
            nc.sync.dma_start(vt[:], pap[sl, :])
            v.append(vt)

            if mode == "dma":
                nc.sync.dma_start(oap[sl, :], v[7][:])
                return

            s2 = stat_pool.tile([P, N_SRC], f32, tag="s2", name=f"s2_{t}")
            dot = stat_pool.tile([P, N_SRC], f32, tag="dot", name=f"dot_{t}")
            for n in range(N_SRC):
                sq_scr = scr_pool.tile([P, D], f32, tag="sq", name=f"sq_{t}_{n}")
                nc.scalar.activation(
                    sq_scr[:], v[n][:], Act.Square, accum_out=s2[:, n : n + 1]
                )
                pr_scr = scr_pool.tile([P, D], f32, tag="pr", name=f"pr_{t}_{n}")
                nc.vector.scalar_tensor_tensor(
                    out=pr_scr[:],
                    in0=v[n][:],
                    scalar=1.0,
                    in1=wn_sb[:],
                    op0=Alu.mult,
                    op1=Alu.mult,
                    accum_out=dot[:, n : n + 1],
                )

            # ms = s2/D + eps ; rs = rsqrt(ms) via bit trick + 3 Newton steps
            ms = stat_pool.tile([P, N_SRC], f32, tag="ms", name=f"ms_{t}")
            nc.vector.tensor_scalar(ms[:], s2[:], 1.0 / D, EPS, Alu.mult, Alu.add)
            ti = stat_pool.tile([P, N_SRC], i32, tag="ti", name=f"ti_{t}")
            nc.vector.tensor_single_scalar(
                ti[:], ms[:].bitcast(i32), 1, Alu.logical_shift_right
            )
            y = stat_pool.tile([P, N_SRC], f32, tag="y", name=f"y_{t}")
            # y_bits = MAGIC - ti  ==  (ti ^ -1) + (MAGIC + 1); walrus refuses
            # mixed bitwise+arith in one tensor_scalar, so two instructions.
            nc.vector.tensor_single_scalar(ti[:], ti[:], -1, Alu.bitwise_xor)
            nc.vector.tensor_single_scalar(y[:].bitcast(i32), ti[:], MAGIC_P1, Alu.add)
            for it in range(3):
                a = stat_pool.tile([P, N_SRC], f32, tag="nra", name=f"nra_{t}_{it}")
                nc.vector.tensor_tensor(a[:], y[:], y[:], Alu.mult)
                nc.vector.tensor_tensor(a[:], a[:], ms[:], Alu.mult)
                nc.vector.tensor_scalar(a[:], a[:], -0.5, 1.5, Alu.mult, Alu.add)
                nc.vector.tensor_tensor(y[:], y[:], a[:], Alu.mult)

            lg = stat_pool.tile([P, N_SRC], f32, tag="lg", name=f"lg_{t}")
            nc.vector.tensor_tensor(lg[:], dot[:], y[:], Alu.mult)

            nm = stat_pool.tile([P, 1], f32, tag="nm", name=f"nm_{t}")
            nc.vector.tensor_reduce(
                nm[:], lg[:], axis=mybir.AxisListType.X, op=Alu.max, negate=True
            )
            e = stat_pool.tile([P, N_SRC], f32, tag="e", name=f"e_{t}")
            den = stat_pool.tile([P, 1], f32, tag="den", name=f"den_{t}")
            nc.scalar.activation(e[:], lg[:], Act.Exp, bias=nm[:], accum_out=den[:])
            r = stat_pool.tile([P, 1], f32, tag="r", name=f"r_{t}")
            nc.vector.reciprocal(r[:], den[:])
            wgt = stat_pool.tile([P, N_SRC], f32, tag="wgt", name=f"wgt_{t}")
            nc.vector.tensor_single_scalar(wgt[:], e[:], r[:], Alu.mult)

            # ---- weighted sum ----
            acc = out_pool.tile([P, D], f32, tag="acc", name=f"acc_{t}")

            if n_pe:
                diags = []
                for j, n in enumerate(pe_src):
                    dg = diag_pool.tile([P, P], f32, tag=f"dg{j}", name=f"dg{j}_{t}")
                    nc.scalar.activation(
                        dg[:], ident_sb[:], Act.Copy, scale=wgt[:, n : n + 1]
                    )
                    diags.append(dg)
                ps = psum_pool.tile([P, D], f32, tag="ps", name=f"ps_{t}")
                half = D // 2
                for h in range(2):
                    cs = slice(h * half, (h + 1) * half)
                    for j, n in enumerate(pe_src):
                        nc.tensor.matmul(
                            ps[:, cs],
                            lhsT=diags[j][:],
                            rhs=v[n][:, cs],
                            start=(j == 0),
                            stop=(j == len(pe_src) - 1),
                        )
                # first DVE FMA consumes the PSUM accumulation
                n0 = dve_src[0]
                nc.vector.scalar_tensor_tensor(
                    out=acc[:],
                    in0=v[n0][:],
                    scalar=wgt[:, n0 : n0 + 1],
                    in1=ps[:],
                    op0=Alu.mult,
                    op1=Alu.add,
                )
                rest = dve_src[1:]
            else:
                n0 = dve_src[0]
                nc.vector.tensor_single_scalar(
                    acc[:], v[n0][:], wgt[:, n0 : n0 + 1], Alu.mult
                )
                rest = dve_src[1:]

            for n in rest:
                nc.vector.scalar_tensor_tensor(
                    out=acc[:],
                    in0=v[n][:],
                    scalar=wgt[:, n : n + 1],
                    in1=acc[:],
                    op0=Alu.mult,
                    op1=Alu.add,
                )

            nc.sync.dma_start(oap[sl, :], acc[:])

        if repeat == 1:
            for t in range(TILES_PER_CORE):
                tile_body(t)
        elif loop:
            # benchmark mode: re-run the whole per-core computation `repeat`
            # times inside a hardware loop so marginal wall-clock isolates
            # on-device execution time from axon/PJRT dispatch overhead.
            with tc.For_i(0, repeat, 1):
                for t in range(TILES_PER_CORE):
                    tile_body(t)
        else:
            # unrolled benchmark mode: no loop back-edge barrier, so repeats
            # pipeline into each other (steady-state throughput measurement).
            for r in range(repeat):
                for t in range(TILES_PER_CORE):
                    tile_body(t, r)

    return _split_multi_waits(nc)


def _get_state():
    if "nc" not in _STATE:
        _STATE["nc"] = _build_nc(N_PE)
    return _STATE["nc"]


def _prepare_in_maps(blocks, partial, norm_weight, w):
    blocks = np.asarray(blocks, dtype=np.float32)
    partial = np.asarray(partial, dtype=np.float32)
    norm_weight = np.asarray(norm_weight, dtype=np.float32)
    w = np.asarray(w, dtype=np.float32)

    wn = (w * norm_weight).astype(np.float32)
    wn_b = np.ascontiguousarray(np.broadcast_to(wn, (P, D)))
    ident = np.eye(P, dtype=np.float32)

    blocks_f = blocks.reshape(N_BLOCKS, TOK_TOTAL, D)
    partial_f = partial.reshape(TOK_TOTAL, D)

    in_maps = []
    for c in range(N_CORES):
        sl = slice(c * TOK_PER_CORE, (c + 1) * TOK_PER_CORE)
        in_maps.append(
            {
                "blocks": np.ascontiguousarray(blocks_f[:, sl, :]),
                "partial": np.ascontiguousarray(partial_f[sl, :]),
                "wn": wn_b,
                "ident": ident,
            }
        )
    return in_maps


def _run(inputs, trace=False, **kwargs):
    from concourse.bass_utils import run_bass_kernel_spmd

    nc = _get_state()
    in_maps = _prepare_in_maps(**inputs)
    bkr = run_bass_kernel_spmd(
        nc, in_maps, core_ids=list(range(N_CORES)), trace=trace, **kwargs
    )
    out = np.concatenate([bkr.results[c]["out"] for c in range(N_CORES)], axis=0)
    return out.reshape(B, T, D), bkr


def kernel(**inputs) -> np.ndarray:
    out, _ = _run(inputs, trace=False)
    return out
